# revision 32
# baseline (speedup 1.0000x reference)
"""Trainium2 Bass kernel for nn_EnhancedFreqLCBlock.

Self-contained: accepts FULL inputs, returns FULL output.
Sharding: 8 cores = 2 batches x 4 quadrant Mamba blocks (expert parallel).
Each core: mask -> quadrant 2D-DCT -> channel LN -> Mamba (hardware
tensor_tensor_scan recurrence) -> residual -> quadrant IDCT contribution.
Host sums the 4 quadrant contributions per batch.
"""
import numpy as np

B, C, H, W = 2, 96, 128, 128
HQ, WQ = H // 2, W // 2          # 64, 64
L = HQ * WQ                      # 4096
D = 192                          # d_inner
S = 16                           # d_state
RK = 6                           # dt_rank
KCONV = 4
NCHUNK = 8
LC = L // NCHUNK                 # 512
NT = (D * S) // 128              # 24 scan partition-tiles
DA, DB = 128, 64                 # d split 192 = 128 + 64
CB = 16                          # channel-block for pipelined load/DCT

_BUILT = {}


def _dct_mat(N):
    n = np.arange(N)
    M = np.cos(np.pi * (2 * n[None, :] + 1) * n[:, None] / (2 * N)) * np.sqrt(2.0 / N)
    M[0] *= 1.0 / np.sqrt(2.0)
    return M.astype(np.float32)


def _build_nc():
    import concourse.bacc as bacc
    import concourse.bass as bass
    import concourse.mybir as mybir
    import concourse.tile as tile

    f32 = mybir.dt.float32
    f32r = mybir.dt.float32r
    bf16 = mybir.dt.bfloat16
    AF = mybir.ActivationFunctionType
    OP = mybir.AluOpType
    AX = mybir.AxisListType
    ts = bass.ts

    nc = bacc.Bacc()

    # ---------------- DRAM I/O ----------------
    xb = nc.dram_tensor("xb", [C, H, W], f32, kind="ExternalInput")
    d_mhqT = nc.dram_tensor("mhqT", [H, HQ], f32, kind="ExternalInput")
    d_mwqT = nc.dram_tensor("mwqT", [W, WQ], f32, kind="ExternalInput")
    d_mhq = nc.dram_tensor("mhq", [HQ, H], f32, kind="ExternalInput")
    d_mwq = nc.dram_tensor("mwq", [WQ, W], f32, kind="ExternalInput")
    d_ident = nc.dram_tensor("ident", [128, 128], f32, kind="ExternalInput")
    d_inwT = nc.dram_tensor("inwT", [C, 2 * D], bf16, kind="ExternalInput")
    d_biasi = nc.dram_tensor("biasi", [D, 1], f32, kind="ExternalInput")
    d_biasz = nc.dram_tensor("biasz", [D, 1], f32, kind="ExternalInput")
    d_convw = nc.dram_tensor("convw", [D, KCONV], f32, kind="ExternalInput")
    d_convb = nc.dram_tensor("convb", [D, 1], f32, kind="ExternalInput")
    d_xpwT = nc.dram_tensor("xpwT", [D, 80], bf16, kind="ExternalInput")
    d_dtwT = nc.dram_tensor("dtwT", [RK, D], bf16, kind="ExternalInput")
    d_dtb = nc.dram_tensor("dtb", [D, 1], f32, kind="ExternalInput")
    d_acol = nc.dram_tensor("acol", [128, NT], f32, kind="ExternalInput")
    d_dp = nc.dram_tensor("dp", [D, 1], f32, kind="ExternalInput")
    d_outwT = nc.dram_tensor("outwT", [D, C], bf16, kind="ExternalInput")
    d_p01grp = nc.dram_tensor("p01grp", [64, 8 * 128], bf16, kind="ExternalInput")
    d_s01 = nc.dram_tensor("s01", [S, 128], bf16, kind="ExternalInput")
    d_r01all = nc.dram_tensor("r01all", [128, 128 * 16], bf16, kind="ExternalInput")
    d_r01ball = nc.dram_tensor("r01ball", [128, 64 * 8], bf16, kind="ExternalInput")
    contrib = nc.dram_tensor("contrib", [C, H, W], f32, kind="ExternalOutput")

    with tile.TileContext(nc) as tc:
        consts = tc.alloc_tile_pool(name="consts", bufs=1)

        # small consts on the gpsimd (SWDGE) queue; big scan-only consts on
        # the scalar queue AFTER the xh blocks (emitted later)
        def cload(dram, shape, dt=f32, eng=None):
            t = consts.tile(shape, dt, name=f"c_{dram.name}")
            s = dram[:].bitcast(f32r) if dt == f32r else dram[:]
            (eng or nc.gpsimd).dma_start(t[:], s)
            return t
        # note: gpsimd DMA casts f32 dram -> bf16 tile automatically

        def cload2(dram, dt=f32):
            ta = consts.tile([DA] + list(dram.shape[1:]), dt, name=f"cA_{dram.name}")
            nc.gpsimd.dma_start(ta[:], dram[0:DA])
            tb = consts.tile([DB] + list(dram.shape[1:]), dt, name=f"cB_{dram.name}")
            nc.gpsimd.dma_start(tb[:], dram[DA:D])
            return ta, tb

        mhqT = cload(d_mhqT, [H, HQ], bf16)
        mwqT = cload(d_mwqT, [W, WQ], bf16)
        mhq = cload(d_mhq, [HQ, H], f32r)
        mwq64 = consts.tile([128, W], f32r, name="c_mwq64")
        nc.gpsimd.dma_start(mwq64[64:128, :], d_mwq[:].bitcast(f32r))
        mwqb = consts.tile([WQ, W], bf16, name="mwqb")
        nc.vector.tensor_copy(mwqb[:], mwq64[64:128, :].bitcast(f32))
        mhqb = consts.tile([HQ, H], bf16, name="mhqb")
        ident = cload(d_ident, [128, 128], f32r)
        nc.vector.tensor_copy(mhqb[:], mhq[:].bitcast(f32))
        identb = consts.tile([C, C], bf16, name="identb")
        nc.vector.tensor_copy(identb[:], ident[0:C, 0:C].bitcast(f32))
        identb128 = consts.tile([128, 128], bf16, name="identb128")
        nc.vector.tensor_copy(identb128[:], ident[:].bitcast(f32))
        inwT = cload(d_inwT, [C, 2 * D], bf16)
        biasiA, biasiB = cload2(d_biasi)
        biaszA, biaszB = cload2(d_biasz)
        convwA, convwB = cload2(d_convw)
        convbA, convbB = cload2(d_convb)
        xpwTA, xpwTB = cload2(d_xpwT, bf16)
        dtwT = cload(d_dtwT, [RK, D], bf16)
        dtbA, dtbB = cload2(d_dtb)
        acol = cload(d_acol, [128, NT])
        dpA, dpB = cload2(d_dp)
        outwTA, outwTB = cload2(d_outwT, bf16)
        ones96f = consts.tile([C, 1], f32)
        nc.vector.memset(ones96f[:], 1.0)
        ones96 = consts.tile([C, 1], f32r)
        nc.vector.tensor_copy(ones96[:], ones96f[:])
        onesr = consts.tile([1, 128], f32)
        nc.vector.memset(onesr[:], 1.0)
        eps64 = consts.tile([WQ, 1], f32)
        nc.vector.memset(eps64[:], 1e-5)
        # depthwise-conv taps as diagonal matrices (PE conv)
        diagA = consts.tile([DA, KCONV * DA], bf16, name="diagA")
        diagB = consts.tile([DB, KCONV * DB], bf16, name="diagB")
        for k in range(KCONV):
            nc.vector.tensor_scalar_mul(diagA[:, ts(k, DA)], identb128[:],
                                        convwA[:, k:k + 1])
            nc.vector.tensor_scalar_mul(diagB[:, ts(k, DB)],
                                        identb128[0:DB, 0:DB],
                                        convwB[:, k:k + 1])

        # persistent psum pools (8 banks total: 3 + 3 + 2)
        pmm = tc.alloc_tile_pool(name="pmm", bufs=3, space="PSUM")
        ppy = tc.alloc_tile_pool(name="ppy", bufs=1, space="PSUM")
        ptp = tc.alloc_tile_pool(name="ptp", bufs=2, space="PSUM")

        def mmtile(p, n, nm):
            return pmm.tile([p, n], f32, name=nm, tag="mm")

        def tptile(p, n, nm, dt=f32):
            return ptp.tile([p, n], dt, name=nm, tag="tp")

        # =============== Phase A: load + mask ===============
        pD_ = tc.alloc_tile_pool(name="pD", bufs=1)
        pB = tc.alloc_tile_pool(name="pB", bufs=1)
        pXH = tc.alloc_tile_pool(name="pXH", bufs=1)
        pA = tc.alloc_tile_pool(name="pA", bufs=1)
        xc = pA.tile([C, H * W], f32r)
        xbf = xb.rearrange("c h w -> c (h w)")
        cpos = (H // 2) * W + (W // 2)
        # center column first so the mask matmuls can start immediately
        nc.sync.dma_start(xc[:, cpos:cpos + 1], xbf[:, cpos:cpos + 1].bitcast(f32r))
        nc.sync.dma_start(xc[:, 0:2048], xbf[:, 0:2048].bitcast(f32r))
        nc.sync.dma_start(xc[:, 2048:8192], xbf[:, 2048:8192].bitcast(f32r))
        nc.sync.dma_start(xc[:, 8192:cpos], xbf[:, 8192:cpos].bitcast(f32r))
        nc.sync.dma_start(xc[:, cpos + 1:], xbf[:, cpos + 1:].bitcast(f32r))
        # xh loaded by channel blocks (scalar + pool queues), h on partitions
        xh = pXH.tile([H, C * W], bf16)
        xb_h = xb.rearrange("c h w -> h c w")
        xh3 = xh.rearrange("h (c w) -> h c w", c=C)
        NBLK = C // CB  # 6 blocks of 16 channels
        for i in range(NBLK):
            nc.gpsimd.dma_start(xh3[:, ts(i, CB), :],
                                xb_h[:, ts(i, CB), :])
        # big scan-only consts after the xh blocks, still SWDGE (doesn't
        # occupy a compute engine during the transfer)
        p01grp = consts.tile([128, 8 * 128], bf16, name="p01grp")
        nc.gpsimd.dma_start(p01grp[0:64, :], d_p01grp[:])
        nc.gpsimd.dma_start(p01grp[64:128, :], d_p01grp[:])
        s01p = consts.tile([80, 128], bf16, name="s01p")
        nc.gpsimd.dma_start(s01p[32:48, :], d_s01[:])
        nc.gpsimd.dma_start(s01p[64:80, :], d_s01[:])
        r01all = cload(d_r01all, [128, 128 * 16], bf16)
        r01ball = cload(d_r01ball, [128, 64 * 8], bf16)

        center = xc[:, cpos:cpos + 1]                       # [96,1]
        cn_ps = tptile(1, 1, "cn_ps")
        nc.tensor.matmul(cn_ps[:], center.bitcast(f32), center.bitcast(f32),
                         start=True, stop=True)
        s049 = pA.tile([1, 1], f32)
        nc.vector.tensor_scalar_mul(s049[:], cn_ps[:], 0.49)
        s049p = tptile(128, 1, "s049p")
        nc.tensor.matmul(s049p[:], onesr[:], s049[:], start=True, stop=True)
        s049b = pA.tile([128, 1], f32)
        nc.vector.tensor_copy(s049b[:], s049p[:])

        num_hw = pA.tile([128, 128], f32)
        ssq_hw = pA.tile([128, 128], f32)
        pSt = tc.alloc_tile_pool(name="pSt", bufs=2)
        for i2 in range(16):
            nrow = pSt.tile([1, 2 * LC], f32, name="nrow")
            srow = pSt.tile([1, 2 * LC], f32, name="srow")
            for h in range(2):
                i = 2 * i2 + h
                nps = mmtile(1, LC, "nps")
                nc.tensor.matmul(nps[:], center, xc[:, ts(i, LC)],
                                 start=True, stop=True)
                if i % 2 == 0:
                    nc.scalar.activation(nrow[:, ts(h, LC)], nps[:], AF.Copy)
                else:
                    nc.vector.tensor_copy(nrow[:, ts(h, LC)], nps[:])
                nc.vector.tensor_tensor(xc[:, ts(i, LC)], xc[:, ts(i, LC)],
                                        xc[:, ts(i, LC)], op=OP.mult)
                sps = tptile(1, LC, "sps")
                nc.tensor.matmul(sps[:], ones96[:], xc[:, ts(i, LC)],
                                 start=True, stop=True)
                if i % 2 == 0:
                    nc.vector.tensor_copy(srow[:, ts(h, LC)], sps[:])
                else:
                    nc.scalar.activation(srow[:, ts(h, LC)], sps[:], AF.Copy)
            nc.sync.dma_start(num_hw[ts(i2, 8), :], nrow[:])
            nc.sync.dma_start(ssq_hw[ts(i2, 8), :], srow[:])
        pSt.release()

        thr = pA.tile([128, 128], f32)
        nc.scalar.activation(thr[:], ssq_hw[:], AF.Sqrt, bias=0.0, scale=s049b[:])
        nc.vector.tensor_scalar_add(thr[:], thr[:], 0.7e-6)
        mask_hw = pD_.tile([128, 128], f32, name="mask_hw")
        nc.vector.tensor_tensor(mask_hw[:], num_hw[:], thr[:], op=OP.is_ge)
        pA.release()

        # ======= Phase B: mask-apply + forward DCT, per channel block =======
        # t2[w, c, hq] = sum_h x[h, c, w] * Mh_q[hq, h]
        t2 = pB.tile([W, C * HQ], bf16)
        t2_3 = t2.rearrange("p (c q) -> p c q", c=C)
        smu = pD_.tile([WQ, HQ], f32, name="smu")
        nc.vector.memset(smu[:], 0.0)
        ssq2 = pD_.tile([WQ, HQ], f32, name="ssq2")
        nc.vector.memset(ssq2[:], 0.0)
        xdqZ = pD_.tile([HQ, C * HQ], bf16)
        xdq3 = xdqZ.rearrange("p (c q) -> p c q", c=C)
        xdq2 = xdqZ[:, :]
        Zb = pD_.tile([HQ, C * HQ], bf16, name="Zb")
        Zb3 = Zb.rearrange("p (c q) -> p c q", c=C)
        XPC = LC // HQ  # 8 channels per xps chunk
        for i in range(NBLK):
            nc.vector.tensor_tensor(
                 xh3[:, ts(i, CB), :], xh3[:, ts(i, CB), :],
                 mask_hw[:, None, :].broadcast_to([128, CB, 128]), op=OP.mult)
            for c in range(i * CB, (i + 1) * CB):
                tps = tptile(W, HQ, "tps")
                nc.tensor.matmul(tps[:], xh3[:, c, :], mhqT[:],
                                 start=True, stop=True)
                nc.any.tensor_copy(t2_3[:, c, :], tps[:])
            for k in range(i * CB // XPC, (i + 1) * CB // XPC):
                xps = mmtile(WQ, LC, "xps")
                nc.tensor.matmul(xps[:], mwqT[:], t2[:, ts(k, LC)],
                                 start=True, stop=True)
                nc.any.tensor_copy(xdq2[:, ts(k, LC)], xps[:])
            # partial LayerNorm stats for this channel block
            blkc = xdq2[:, ts(i, CB * HQ)]
            blk3t = xdq3[:, ts(i, CB), :].transpose([0, 2, 1])
            pmu = pB.tile([WQ, HQ], f32, name="pmu", tag="pmu", bufs=2)
            nc.vector.tensor_reduce(pmu[:], blk3t, axis=AX.X,
                                    op=OP.add)
            nc.vector.tensor_tensor(smu[:], smu[:], pmu[:], op=OP.add)
            sqs = pB.tile([WQ, CB * HQ], bf16, name="sqs", tag="sqs", bufs=2)
            nc.vector.tensor_tensor(sqs[:], blkc, blkc, op=OP.mult)
            psq = pB.tile([WQ, HQ], f32, name="psq", tag="psq", bufs=2)
            nc.vector.tensor_reduce(
                psq[:], sqs.rearrange("p (c q) -> p c q", c=CB).transpose(
                    [0, 2, 1]), axis=AX.X, op=OP.add)
            nc.vector.tensor_tensor(ssq2[:], ssq2[:], psq[:], op=OP.add)
        pXH.release()
        pB.release()

        # =============== Phase C: LayerNorm over c ===============
        pG = tc.alloc_tile_pool(name="pG", bufs=1)
        pF = tc.alloc_tile_pool(name="pF", bufs=1)
        pE = tc.alloc_tile_pool(name="pE", bufs=1)
        pC = tc.alloc_tile_pool(name="pC", bufs=1)
        xn = pC.tile([WQ, C * HQ], bf16)
        mu = pC.tile([WQ, HQ], f32)
        nc.vector.tensor_scalar_mul(mu[:], smu[:], 1.0 / C)
        var = pC.tile([WQ, HQ], f32)
        nc.vector.tensor_scalar_mul(ssq2[:], ssq2[:], 1.0 / C)
        nc.vector.tensor_tensor(var[:], mu[:], mu[:], op=OP.mult)
        nc.vector.tensor_tensor(var[:], ssq2[:], var[:], op=OP.subtract)
        sd = pC.tile([WQ, HQ], f32)
        nc.scalar.activation(sd[:], var[:], AF.Sqrt, bias=eps64[:])
        inv = pC.tile([WQ, HQ], f32)
        nc.vector.reciprocal(inv[:], sd[:])
        nmu = pC.tile([WQ, HQ], f32)
        nc.vector.tensor_tensor(nmu[:], mu[:], inv[:], op=OP.mult)
        nc.vector.tensor_scalar_mul(nmu[:], nmu[:], -1.0)
        xn3 = xn.rearrange("p (c q) -> p c q", c=C)
        xn_c = pE.tile([C, L], bf16)

        def ln_block(b8):
            bs = bass.ds(8 * b8, 8)
            nc.vector.tensor_tensor(
                xn3[:, :, bs], xdq3[:, :, bs],
                inv[:, None, 8 * b8:8 * b8 + 8].broadcast_to([WQ, C, 8]),
                op=OP.mult)
            nc.vector.tensor_tensor(
                xn3[:, :, bs], xn3[:, :, bs],
                nmu[:, None, 8 * b8:8 * b8 + 8].broadcast_to([WQ, C, 8]),
                op=OP.add)
            for hq in range(8 * b8, 8 * b8 + 8):
                tps2 = tptile(C, WQ, "tps2", bf16)
                nc.tensor.matmul(tps2[:], xn3[:, :, hq],
                                 identb128[0:WQ, 0:WQ],
                                 is_transpose=True, start=True, stop=True)
                nc.any.tensor_copy(xn_c[:, ts(hq, WQ)], tps2[:])

        # ====== Phase D+F interleaved: in_proj/conv then scan per 1024 ======
        pT = tc.alloc_tile_pool(name="pT", bufs=2)
        xiA = pF.tile([DA, KCONV - 1 + L], bf16)
        xiB = pF.tile([DB, KCONV - 1 + L], bf16)
        nc.vector.memset(xiA[:, 0:KCONV - 1], 0.0)
        nc.vector.memset(xiB[:, 0:KCONV - 1], 0.0)
        xi2A = pG.tile([DA, L], bf16)
        zsA = pG.tile([DA, L], bf16)
        xi2B = pG.tile([DB, L], bf16, name="xi2B_t")
        zsB = pG.tile([DB, L], bf16, name="zsB_t")
        hlast = pG.tile([128, NT], bf16)
        zstage = pG.tile([WQ, 16 * C], bf16, name="zstage")
        zstage3 = zstage.rearrange("p (r c) -> p r c", r=16)
        LCF = 2 * LC

        def phaseD_chunk(i):
            ps0 = mmtile(128, LC, "ps0")
            nc.tensor.matmul(ps0[:], inwT[:, 0:128], xn_c[:, ts(i, LC)],
                             start=True, stop=True)
            ps1 = mmtile(128, LC, "ps1")
            nc.tensor.matmul(ps1[:], inwT[:, 128:256], xn_c[:, ts(i, LC)],
                             start=True, stop=True)
            ps2 = mmtile(128, LC, "ps2")
            nc.tensor.matmul(ps2[:], inwT[:, 256:384], xn_c[:, ts(i, LC)],
                             start=True, stop=True)
            o = KCONV - 1 + i * LC
            nc.scalar.activation(xiA[:, o:o + LC], ps0[:], AF.Identity,
                                 bias=biasiA[:])
            nc.scalar.activation(xiB[:, o:o + LC], ps1[0:64, :], AF.Identity,
                                 bias=biasiB[:])
            nc.scalar.activation(zsA[0:64, ts(i, LC)], ps1[64:128, :], AF.Silu,
                                 bias=biaszA[0:64, :])
            nc.scalar.activation(zsA[64:128, ts(i, LC)], ps2[0:64, :], AF.Silu,
                                 bias=biaszA[64:128, :])
            nc.scalar.activation(zsB[:, ts(i, LC)], ps2[64:128, :], AF.Silu,
                                 bias=biaszB[:])
            cvA = mmtile(DA, LC, "cvA")
            cvB = tptile(DB, LC, "cvB")
            for k in range(KCONV):
                nc.tensor.matmul(cvA[:], diagA[:, ts(k, DA)],
                                 xiA[:, i * LC + k:(i + 1) * LC + k],
                                 start=(k == 0), stop=(k == KCONV - 1))
                nc.tensor.matmul(cvB[:], diagB[:, ts(k, DB)],
                                 xiB[:, i * LC + k:(i + 1) * LC + k],
                                 start=(k == 0), stop=(k == KCONV - 1))
            nc.scalar.activation(xi2A[:, ts(i, LC)], cvA[:], AF.Silu,
                                 bias=convbA[:])
            nc.scalar.activation(xi2B[:, ts(i, LC)], cvB[:], AF.Silu,
                                 bias=convbB[:])

        chq = {}
        prept = {}
        ypst = {}

        def prep(i):
            dbl_c = pT.tile([80, LCF], bf16, name="dbl_c")
            for h in range(2):
                off = i * LCF + h * LC
                sl = bass.ds(off, LC)
                dblps = mmtile(80, LC, "dblps")
                nc.tensor.matmul(dblps[:], xpwTA[:], xi2A[:, sl],
                                 start=True, stop=False)
                nc.tensor.matmul(dblps[:], xpwTB[:], xi2B[:, sl],
                                 start=False, stop=True)
                nc.scalar.activation(dbl_c[:, ts(h, LC)], dblps[:], AF.Copy)
            deltaA = pT.tile([DA, LCF], bf16, name="deltaA")
            deltaB = pT.tile([DB, LCF], bf16, name="deltaB")
            for h in range(2):
                dtpA = mmtile(DA, LC, "dtpA")
                nc.tensor.matmul(dtpA[:], dtwT[:, 0:DA], dbl_c[0:RK, ts(h, LC)],
                                 start=True, stop=True)
                nc.scalar.activation(deltaA[:, ts(h, LC)], dtpA[:], AF.Exp,
                                     bias=dtbA[:])
                dtpB = mmtile(DB, LC, "dtpB")
                nc.tensor.matmul(dtpB[:], dtwT[:, DA:D], dbl_c[0:RK, ts(h, LC)],
                                 start=True, stop=True)
                nc.scalar.activation(deltaB[:, ts(h, LC)], dtpB[:], AF.Exp,
                                     bias=dtbB[:])
            nc.scalar.activation(deltaA[:], deltaA[:], AF.Ln, bias=1.0)
            nc.scalar.activation(deltaB[:], deltaB[:], AF.Ln, bias=1.0)
            dXA = pT.tile([DA, LCF], bf16, name="dXA")
            nc.vector.tensor_tensor(dXA[:], deltaA[:],
                                    xi2A[:, ts(i, LCF)], op=OP.mult)
            dXB = pT.tile([DB, LCF], bf16, name="dXB")
            nc.vector.tensor_tensor(dXB[:], deltaB[:],
                                    xi2B[:, ts(i, LCF)], op=OP.mult)
            brep = pT.tile([128, LCF], bf16, name="brep")
            crep = pT.tile([128, LCF], bf16, name="crep")
            for h in range(2):
                brep_ps = mmtile(128, LC, "brep_ps")
                nc.tensor.matmul(brep_ps[:], s01p[32:48, :],
                                 dbl_c[32:48, ts(h, LC)],
                                 start=True, stop=True)
                nc.scalar.activation(brep[:, ts(h, LC)], brep_ps[:], AF.Copy)
                crep_ps = mmtile(128, LC, "crep_ps")
                nc.tensor.matmul(crep_ps[:], s01p[64:80, :],
                                 dbl_c[64:80, ts(h, LC)],
                                 start=True, stop=True)
                nc.scalar.activation(crep[:, ts(h, LC)], crep_ps[:], AF.Copy)
            prept[i] = (deltaA, deltaB, dXA, dXB, brep, crep)

        def emit_r01(i, j):
            ypsA0, ypsA1, ypsBp = ypst[i]
            ch = chq.pop((i, j))
            jj = j if j < 16 else j - 16
            if j < 16:
                nc.tensor.matmul(ypsA0[:], r01all[:, ts(jj, 128)],
                                 ch[:, 0:LC], start=(j == 0), stop=(j == 15))
                nc.tensor.matmul(ypsA1[:], r01all[:, ts(jj, 128)],
                                 ch[:, LC:LCF], start=(j == 0), stop=(j == 15))
            else:
                nc.tensor.matmul(ypsBp[0:DB, :], r01ball[:, ts(jj, 64)],
                                 ch[:, 0:LC], start=(j == 16), stop=(j == 23),
                                 skip_group_check=True)
                nc.tensor.matmul(ypsBp[DB:128, :], r01ball[:, ts(jj, 64)],
                                 ch[:, LC:LCF], start=(j == 16), stop=(j == 23),
                                 skip_group_check=True)

        def zadd(i):
            blk = bass.ds(16 * i, 16)
            eng = nc.vector if i == NCHUNK // 2 - 1 else nc.gpsimd
            eng.tensor_tensor(
                Zb3[:, :, blk].transpose([0, 2, 1]),
                xdq3[:, :, blk].transpose([0, 2, 1]),
                zstage3[:, :, :], op=OP.add)

        def jbody(i, j):
            deltaA, deltaB, dXA, dXB, brep, crep = prept[i]
            jj = j if j < 16 else j - 16
            dsl, xsl = (deltaA, dXA) if j < 16 else (deltaB, dXB)
            g, k = 64 * (jj // 8), jj % 8
            dA_t = pT.tile([128, LCF], bf16, name="dA_t", bufs=3)
            dBu = pT.tile([128, LCF], bf16, name="dBu", bufs=3)
            dxc = None
            if j % 2 == 0:
                dxc = pT.tile([128, LCF], bf16, name="dxc", bufs=2)
            for h in range(2):
                drep = mmtile(128, LC, "drep")
                nc.tensor.matmul(drep[:], p01grp[g:g + 64, ts(k, 128)],
                                 dsl[g:g + 64, ts(h, LC)],
                                 start=True, stop=True)
                nc.scalar.activation(dA_t[:, ts(h, LC)], drep[:], AF.Exp,
                                     scale=acol[:, j:j + 1])
                dxrep = tptile(128, LC, "dxrep")
                nc.tensor.matmul(dxrep[:], p01grp[g:g + 64, ts(k, 128)],
                                 xsl[g:g + 64, ts(h, LC)],
                                 start=True, stop=True)
                if j % 2 == 0:
                    nc.scalar.activation(dxc[:, ts(h, LC)], dxrep[:], AF.Copy)
                else:
                    nc.vector.tensor_tensor(dBu[:, ts(h, LC)], dxrep[:],
                                            brep[:, ts(h, LC)], op=OP.mult)
            if j % 2 == 0:
                nc.vector.tensor_tensor(dBu[:], dxc[:], brep[:], op=OP.mult)
            h_t = pT.tile([128, LCF], bf16, name="h_t", bufs=3)
            init = 0.0 if i == 0 else hlast[:, j:j + 1]
            nc.vector.tensor_tensor_scan(
                h_t[:], dA_t[:], dBu[:], init, op0=OP.mult, op1=OP.add)
            nc.scalar.activation(hlast[:, j:j + 1], h_t[:, LCF - 1:LCF],
                                 AF.Copy)
            ch = pT.tile([128, LCF], bf16, name="ch", bufs=3)
            nc.vector.tensor_tensor(ch[:], h_t[:], crep[:], op=OP.mult)
            chq[(i, j)] = ch

        def tail(i):
            ypsA0, ypsA1, ypsBp = ypst[i]
            ypsB0, ypsB1 = ypsBp[0:DB, :], ypsBp[DB:128, :]
            yA = pT.tile([DA, LCF], bf16, name="yA", bufs=2)
            yB = pT.tile([DB, LCF], bf16, name="yB", bufs=2)
            for h, (ya_ps, yb_ps) in enumerate([(ypsA0[:], ypsB0),
                                                (ypsA1[:], ypsB1)]):
                off = i * LCF + h * LC
                sl = bass.ds(off, LC)
                nc.vector.scalar_tensor_tensor(
                    yA[:, ts(h, LC)], xi2A[:, sl], dpA[:], ya_ps,
                    op0=OP.mult, op1=OP.add)
                nc.vector.scalar_tensor_tensor(
                    yB[:, ts(h, LC)], xi2B[:, sl], dpB[:], yb_ps,
                    op0=OP.mult, op1=OP.add)
            nc.vector.tensor_tensor(yA[:], yA[:], zsA[:, ts(i, LCF)], op=OP.mult)
            nc.vector.tensor_tensor(yB[:], yB[:], zsB[:, ts(i, LCF)], op=OP.mult)
            mout = pT.tile([C, LCF], bf16, name="mout", bufs=2)
            for h in range(2):
                mps = mmtile(C, LC, "mps")
                nc.tensor.matmul(mps[:], outwTA[:], yA[:, ts(h, LC)],
                                 start=True, stop=False)
                nc.tensor.matmul(mps[:], outwTB[:], yB[:, ts(h, LC)],
                                 start=False, stop=True)
                nc.scalar.activation(mout[:, ts(h, LC)], mps[:], AF.Copy)
            for r in range(16):
                rps = tptile(WQ, C, "rps", bf16)
                nc.tensor.matmul(rps[:], mout[:, ts(r, WQ)], identb[:],
                                 is_transpose=True, start=True, stop=True)
                nc.scalar.activation(zstage3[:, r, :], rps[:], AF.Copy)

        ln_block(0)
        phaseD_chunk(0)
        ln_block(1)
        phaseD_chunk(1)
        prep(0)
        for b8 in range(2, 8):
            ln_block(b8)
        NI = NCHUNK // 2
        for i in range(NI):
            ypst[i] = (ppy.tile([128, LC], f32, name="ypsA0", tag="ypsA0"),
                       ppy.tile([128, LC], f32, name="ypsA1", tag="ypsA1"),
                       ppy.tile([128, LC], f32, name="ypsBp", tag="ypsBp"))
            for j in range(NT):
                if i > 0 and j == 2:
                    zadd(i - 1)
                jbody(i, j)
                if j > 1:
                    emit_r01(i, j - 2)
                if i + 1 < NI:
                    if j == 4:
                        phaseD_chunk(2 * i + 2)
                        phaseD_chunk(2 * i + 3)
                    elif j == 12:
                        prep(i + 1)
            emit_r01(i, NT - 2)
            emit_r01(i, NT - 1)
            tail(i)
        zadd(NI - 1)
        pT.release()
        pC.release()
        pE.release()
        pF.release()
        pG.release()

        # =============== Phase G: IDCT contribution (fp32r) ===============
        pH = tc.alloc_tile_pool(name="pH", bufs=1)
        # t7[hq, c, W] = sum_wq Z[wq, c, hq] * Mw_q[wq, W]  (per-c matmul)
        t7 = pH.tile([HQ, C * W], bf16)
        t7_3 = t7.rearrange("p (c w) -> p c w", c=C)
        for c in range(C):
            t7ps = tptile(HQ, W, "t7ps")
            nc.tensor.matmul(t7ps[:], Zb3[:, c, :], mwqb[:],
                             start=True, stop=True)
            if c % 2 == 0:
                nc.scalar.activation(t7_3[:, c, :], t7ps[:], AF.Copy)
            else:
                nc.vector.tensor_copy(t7_3[:, c, :], t7ps[:])
        ctr_h = contrib.rearrange("c h w -> h c w")
        pSo = tc.alloc_tile_pool(name="pSo", bufs=3)
        for i in range(24):
            cps = mmtile(H, LC, "cps")
            nc.tensor.matmul(cps[:], mhqb[:], t7[:, ts(i, LC)], start=True, stop=True)
            csb = pSo.tile([H, LC], f32, name="csb")
            if i % 2 == 0:
                nc.scalar.activation(csb[:], cps[:], AF.Copy)
            else:
                nc.vector.tensor_copy(csb[:], cps[:])
            eng = nc.sync if i % 2 == 0 else nc.scalar
            eng.dma_start(
                ctr_h[:, ts(i, 4), :],
                csb[:].rearrange("h (c w) -> h c w", c=4))
        pSo.release()
        pH.release()
        pD_.release()
        ptp.release()
        ppy.release()
        pmm.release()
        consts.release()

    nc.compile()
    return nc


def _host_inputs(inputs):
    """Build the 8 per-core input maps."""
    x = inputs["x"]
    ln_w, ln_b = inputs["ln_w"], inputs["ln_b"]
    Mh = _dct_mat(H)
    Mw = _dct_mat(W)
    ident = np.eye(128, dtype=np.float32)
    s01 = np.zeros((S, 128), np.float32)
    p01grp = np.zeros((64, 8 * 128), np.float32)
    r01all = np.zeros((128, 128 * 16), np.float32)
    r01ball = np.zeros((128, 64 * 8), np.float32)
    for p in range(128):
        s01[p % S, p] = 1.0
        for k in range(8):
            p01grp[8 * k + p // S, 128 * k + p] = 1.0
        for j in range(16):
            r01all[p, 128 * j + 8 * j + p // S] = 1.0
        for j in range(8):
            r01ball[p, 64 * j + 8 * j + p // S] = 1.0
    in_maps = []
    for k in range(8):
        b, q = k // 4, k % 4
        h0 = (q // 2) * HQ
        w0 = (q % 2) * WQ
        in_w2 = (inputs["in_w"][q] * ln_w[None, :]).astype(np.float32)
        bias_e = (inputs["in_w"][q] @ ln_b).astype(np.float32)
        A = (-np.exp(inputs["A_log"][q])).astype(np.float32)  # [D, S]
        acol = np.zeros((128, NT), np.float32)
        for j in range(NT):
            for p in range(128):
                acol[p, j] = A[j * 8 + p // S, p % S]
        m = {
            "xb": x[b],
            "mhqT": Mh[h0:h0 + HQ, :].T,
            "mwqT": Mw[w0:w0 + WQ, :].T,
            "mhq": Mh[h0:h0 + HQ, :],
            "mwq": Mw[w0:w0 + WQ, :],
            "ident": ident,
            "inwT": in_w2.T,
            "biasi": bias_e[:D, None],
            "biasz": bias_e[D:, None],
            "convw": inputs["conv_w"][q],
            "convb": inputs["conv_b"][q][:, None],
            "xpwT": np.concatenate([
                inputs["xp_w"][q][0:RK].T,
                np.zeros((D, 32 - RK), np.float32),
                inputs["xp_w"][q][RK:RK + S].T,
                np.zeros((D, 16), np.float32),
                inputs["xp_w"][q][RK + S:].T], axis=1),
            "dtwT": inputs["dt_w"][q].T,
            "dtb": inputs["dt_b"][q][:, None],
            "acol": acol,
            "dp": inputs["Dp"][q][:, None],
            "outwT": inputs["out_w"][q].T,
            "p01grp": p01grp,
            "s01": s01,
            "r01all": r01all,
            "r01ball": r01ball,
        }
        import ml_dtypes
        bf = ["inwT", "xpwT", "dtwT", "outwT", "p01grp", "s01",
              "r01all", "r01ball"]
        in_maps.append({
            kk: np.ascontiguousarray(np.asarray(
                vv, ml_dtypes.bfloat16 if kk in bf else np.float32))
            for kk, vv in m.items()})
    return in_maps


def kernel(**inputs):
    from concourse import bass_utils
    inputs = {k: np.asarray(v) for k, v in inputs.items()}
    if "nc" not in _BUILT:
        _BUILT["nc"] = _build_nc()
    nc = _BUILT["nc"]
    in_maps = _host_inputs(inputs)
    res = bass_utils.run_bass_kernel_spmd(nc, in_maps, core_ids=list(range(8)))
    out = np.zeros((B, C, H, W), np.float32)
    for k in range(8):
        out[k // 4] += res.results[k]["contrib"]
    return out


if __name__ == "__main__":
    rng = np.random.default_rng(0)
    demo = {
        "x": rng.standard_normal((B, C, H, W)).astype(np.float32),
        "ln_w": np.ones(C, np.float32), "ln_b": np.zeros(C, np.float32),
        "in_w": (rng.standard_normal((4, 2 * D, C)) * 0.02).astype(np.float32),
        "conv_w": (rng.standard_normal((4, D, KCONV)) * 0.02).astype(np.float32),
        "conv_b": np.zeros((4, D), np.float32),
        "xp_w": (rng.standard_normal((4, RK + 2 * S, D)) * 0.02).astype(np.float32),
        "dt_w": (rng.standard_normal((4, D, RK)) * 0.02).astype(np.float32),
        "dt_b": np.full((4, D), -4.0, np.float32),
        "A_log": np.tile(np.log(np.arange(1, S + 1, dtype=np.float32)), (4, D, 1)),
        "Dp": np.ones((4, D), np.float32),
        "out_w": (rng.standard_normal((4, C, D)) * 0.02).astype(np.float32),
    }
    out = kernel(**demo)
    print("kernel output:", out.shape, out.dtype)
